# revision 16
# baseline (speedup 1.0000x reference)
"""ALiBi causal attention on 8 Trainium2 NeuronCores.

Sharding: tensor-parallel over heads (2 heads/core) for QKV projection and
attention; two batch-split AllToAlls redistribute the (normalized,
transposed) attention outputs so each core owns 256 tokens of each batch
for the output projection. The b0 AllToAll and b0 output projection
overlap with b1's attention compute.

Layout choices (all chosen to avoid on-chip transposes):
  - x is passed host-transposed as xT [D=1024, B*T=4096] in bf16.
  - Q/K are produced in "head-transposed" layout [head_dim, tokens] and
    augmented with one extra contraction row so that the per-query ALiBi
    term -slope*i rides the score matmul (exactly cancelled by softmax,
    so bf16 rounding of it is harmless).
  - Scores are computed transposed: ST[k, q] = K'.T-block @ Q', so the
    softmax reduction (over k) aligns with the AV matmul contraction and
    the denominator falls out of a ones-column appended to V.
  - exp via ScalarE with per-partition bias slope*j in exact f32.
  - Causal masking: only the diagonal-intersecting k-block per q-tile
    needs a 128x128 triangular min-clamp; fully-masked columns are never
    computed or streamed.

Tiles are deliberately small/chunked (xT per [k,512-token] block, Q/K per
[head, 512-token] chunk, V per [token-block]) because Tile's dependency
tracking is per-tile: chunking lets attention start while later
projections still run, and projections start after the first DMA chunk.
DMA queues: xT streams on the sync queue; weights/constants go on the
scalar queue so they don't delay the first projection matmuls.
"""

import sys

if "/opt/trn_rl_repo" not in sys.path:
    sys.path.insert(0, "/opt/trn_rl_repo")

import numpy as np
import ml_dtypes

import concourse.bass as bass
import concourse.bacc as bacc
import concourse.tile as tile
import concourse.mybir as mybir
from concourse import bass_utils

BF16 = mybir.dt.bfloat16
F32 = mybir.dt.float32
NPBF16 = ml_dtypes.bfloat16

B, T, D = 2, 2048, 1024
H, HD = 16, 64
NC = 8
HPC = H // NC          # heads per core = 2
TOK = B * T            # 4096
TPC = TOK // NC        # tokens per core after a2a = 512 (256 per batch)
NKB = T // 128         # 16 k-blocks per sequence
NQT = T // 512         # 4 q-tiles per sequence
NTC = TOK // 512       # 8 token-chunks of 512
KAUG = HD + 1          # 65: head_dim + 1 aug row

_COMPILED = None


def _build():
    nc = bacc.Bacc("TRN2", target_bir_lowering=False, debug=False, num_devices=NC)

    xT_d = nc.dram_tensor("xT", [D, TOK], BF16, kind="ExternalInput")
    wq_d = nc.dram_tensor("wq", [D, 128], BF16, kind="ExternalInput")
    wk_d = nc.dram_tensor("wk", [D, 128], BF16, kind="ExternalInput")
    wv_d = nc.dram_tensor("wv", [D, 128], BF16, kind="ExternalInput")
    wo_d = nc.dram_tensor("wo", [D, D], BF16, kind="ExternalInput")
    aug_d = nc.dram_tensor("aug", [HPC + 1, T], BF16, kind="ExternalInput")
    kbias_d = nc.dram_tensor("kbias", [128, HPC * NKB], F32, kind="ExternalInput")
    cap_d = nc.dram_tensor("cap", [128, 128], F32, kind="ExternalInput")
    ind_d = nc.dram_tensor("ind", [1, 256], BF16, kind="ExternalInput")
    out_d = nc.dram_tensor("out", [TPC, D], F32, kind="ExternalOutput")
    ccin = [
        nc.dram_tensor(f"ccin{b}", [NC * 128, TPC // B], BF16, kind="Internal")
        for b in range(B)
    ]
    ccout = [
        nc.dram_tensor(f"ccout{b}", [NC * 128, TPC // B], BF16, kind="Internal")
        for b in range(B)
    ]

    with tile.TileContext(nc) as tc:
        with (
            tc.tile_pool(name="const", bufs=1) as cpool,
            tc.tile_pool(name="work", bufs=1) as wpool,
            tc.tile_pool(name="ps", bufs=4, space="PSUM") as ps,
            tc.tile_pool(name="psot", bufs=4, space="PSUM") as psot,
        ):
            # ---- xT: one tile per (k-chunk, token-chunk), sync queue --
            xt = [[None] * NTC for _ in range(8)]
            for tc8 in range(NTC):
                for k in range(8):
                    t_ = cpool.tile([128, 512], BF16, name=f"xt{k}_{tc8}", tag=f"xt{k}_{tc8}")
                    nc.sync.dma_start(t_[:], xT_d[128 * k : 128 * (k + 1), 512 * tc8 : 512 * (tc8 + 1)])
                    xt[k][tc8] = t_

            # ---- weights + constants on the scalar DMA queue ----------
            wq_t = cpool.tile([128, D], BF16, name="wq_t", tag="wq_t")
            wk_t = cpool.tile([128, D], BF16, name="wk_t", tag="wk_t")
            wv_t = cpool.tile([128, D], BF16, name="wv_t", tag="wv_t")
            for k in range(8):
                nc.scalar.dma_start(wq_t[:, 128 * k : 128 * (k + 1)], wq_d[128 * k : 128 * (k + 1), :])
                nc.scalar.dma_start(wk_t[:, 128 * k : 128 * (k + 1)], wk_d[128 * k : 128 * (k + 1), :])
                nc.scalar.dma_start(wv_t[:, 128 * k : 128 * (k + 1)], wv_d[128 * k : 128 * (k + 1), :])
            kbias_t = cpool.tile([128, HPC * NKB], F32, name="kbias_t", tag="kbias_t")
            nc.scalar.dma_start(kbias_t[:], kbias_d[:])
            cap_t = cpool.tile([128, 128], F32, name="cap_t", tag="cap_t")
            nc.scalar.dma_start(cap_t[:], cap_d[:])
            ind_t = cpool.tile([1, 256], BF16, name="ind_t", tag="ind_t")
            nc.scalar.dma_start(ind_t[:], ind_d[:])

            # QTa/KTa: per (b, hl, chunk-of-512): [65, 512]; row 64 = aug.
            qta = [[[None] * NQT for _ in range(HPC)] for _ in range(B)]
            kta = [[[None] * NQT for _ in range(HPC)] for _ in range(B)]
            for b in range(B):
                for hl in range(HPC):
                    for c in range(NQT):
                        q_ = cpool.tile([KAUG, 512], BF16, name=f"qta{b}{hl}{c}", tag=f"qta{b}{hl}{c}")
                        k_ = cpool.tile([KAUG, 512], BF16, name=f"kta{b}{hl}{c}", tag=f"kta{b}{hl}{c}")
                        nc.scalar.dma_start(q_[64:65, :], aug_d[hl : hl + 1, 512 * c : 512 * (c + 1)])
                        nc.scalar.dma_start(k_[64:65, :], aug_d[HPC : HPC + 1, 512 * c : 512 * (c + 1)])
                        qta[b][hl][c] = q_
                        kta[b][hl][c] = k_
            # V: per (b, k-block): [128, 130]: 64 cols head A, ones col,
            # 64 cols head B, ones col.
            vt = [[None] * NKB for _ in range(B)]
            for b in range(B):
                for kb in range(NKB):
                    v_ = cpool.tile([128, 130], BF16, name=f"v{b}_{kb}", tag=f"v{b}_{kb}")
                    nc.vector.memset(v_.rearrange("p (a c) -> p a c", c=65)[:, :, 64], 1.0)
                    vt[b][kb] = v_

            # ---- phase 1: QKV projections (chunk-interleaved) ---------
            def qkv_chunk(tc8):
                b, cq = tc8 // NQT, tc8 % NQT
                for w_t, dsts in ((wq_t, qta), (wk_t, kta)):
                    pp = ps.tile([128, 512], F32, name="pp", tag="mm512")
                    for k in range(8):
                        nc.tensor.matmul(
                            pp[:],
                            w_t[:, 128 * k : 128 * (k + 1)],
                            xt[k][tc8][:],
                            start=(k == 0),
                            stop=(k == 7),
                        )
                    nc.vector.tensor_copy(dsts[b][0][cq][0:64, :], pp[0:64, :])
                    nc.vector.tensor_copy(dsts[b][1][cq][0:64, :], pp[64:128, :])
                for j in range(4):
                    kb = 4 * cq + j
                    pv = psot.tile([128, 128], F32, name="pv", tag="otv")
                    for k in range(8):
                        nc.tensor.matmul(
                            pv[:],
                            xt[k][tc8][:, 128 * j : 128 * (j + 1)],
                            wv_t[:, 128 * k : 128 * (k + 1)],
                            start=(k == 0),
                            stop=(k == 7),
                        )
                    nc.vector.tensor_copy(vt[b][kb][:, 0:64], pv[:, 0:64])
                    nc.vector.tensor_copy(vt[b][kb][:, 65:129], pv[:, 64:128])

            # ---- phase 2: attention for one (b, q-tile) ---------------
            def attn_group(b, qt):
                nkb = 4 * qt + 4
                ots = [
                    psot.tile([KAUG, 512], F32, name="ot", tag="otv")
                    for _ in range(HPC)
                ]
                for kb in range(nkb):
                    off = max(0, 128 * (kb - 4 * qt))
                    for hl in range(HPC):
                        sc = ps.tile([128, 512], F32, name="sc", tag="mm512")
                        nc.tensor.matmul(
                            sc[:, off:512],
                            kta[b][hl][kb // 4][:, 128 * (kb % 4) : 128 * (kb % 4 + 1)],
                            qta[b][hl][qt][:, off:512],
                            start=True,
                            stop=True,
                        )
                        if kb >= 4 * qt:
                            nc.vector.tensor_tensor(
                                sc[:, off : off + 128],
                                sc[:, off : off + 128],
                                cap_t[:],
                                mybir.AluOpType.min,
                            )
                        ex = wpool.tile([128, 512], BF16, name="ex", tag="ex", bufs=6)
                        nc.scalar.activation(
                            ex[:, off:512],
                            sc[:, off:512],
                            mybir.ActivationFunctionType.Exp,
                            bias=kbias_t[:, NKB * hl + kb : NKB * hl + kb + 1],
                            scale=0.125,
                        )
                        nc.tensor.matmul(
                            ots[hl][:, off:512],
                            vt[b][kb][:, 65 * hl : 65 * hl + 65],
                            ex[:, off:512],
                            start=(kb == 0),
                            stop=(kb == nkb - 1),
                        )
                dena = wpool.tile([1, 512], BF16, name="dena", tag="dena", bufs=2)
                denb = wpool.tile([1, 512], BF16, name="denb", tag="denb", bufs=2)
                nc.vector.tensor_copy(dena[:], ots[0][64:65, :])
                nc.vector.tensor_copy(denb[:], ots[1][64:65, :])
                bc = ps.tile([128, 512], F32, name="bc", tag="mm512")
                nc.tensor.matmul(bc[:], ind_t[:, 0:128], dena[:], start=True, stop=False)
                nc.tensor.matmul(bc[:], ind_t[:, 128:256], denb[:], start=False, stop=True)
                bcs = wpool.tile([128, 512], F32, name="bcs", tag="bcs", bufs=2)
                nc.vector.tensor_copy(bcs[:], bc[:])
                bci = wpool.tile([128, 512], F32, name="bci", tag="bci", bufs=2)
                nc.vector.reciprocal_approx_fast(bci[:], bcs[:])
                otn = wpool.tile([128, 512], BF16, name="otn", tag="otn", bufs=3)
                nc.vector.tensor_tensor(
                    otn[0:64, :], ots[0][0:64, :], bci[0:64, :], mybir.AluOpType.mult
                )
                nc.vector.tensor_tensor(
                    otn[64:128, :], ots[1][0:64, :], bci[64:128, :], mybir.AluOpType.mult
                )
                # two destination blocks of 256 tokens each
                for half in range(2):
                    blk = 2 * qt + half
                    nc.sync.dma_start(
                        ccin[b][128 * blk : 128 * (blk + 1), :],
                        otn[:, 256 * half : 256 * (half + 1)],
                    )

            # ---- phase 4: output projection for one batch -------------
            at = [[None] * 8 for _ in range(B)]

            def yrecv(b):
                for k in range(8):
                    a_ = cpool.tile([128, TPC // B], BF16, name=f"at{b}_{k}", tag=f"at{b}_{k}")
                    nc.sync.dma_start(a_[:], ccout[b][128 * k : 128 * (k + 1), :])
                    at[b][k] = a_

            def ypiece(b, tb, n):
                yp = ps.tile([128, 512], F32, name="yp", tag="mm512")
                for k in range(8):
                    nc.tensor.matmul(
                        yp[:],
                        at[b][k][:, 128 * tb : 128 * (tb + 1)],
                        wo_t[:, D * k + 512 * n : D * k + 512 * (n + 1)],
                        start=(k == 0),
                        stop=(k == 7),
                    )
                ys = wpool.tile([128, 512], F32, name="ys", tag="ys", bufs=2)
                nc.vector.tensor_copy(ys[:], yp[:])
                nc.sync.dma_start(
                    out_d[256 * b + 128 * tb : 256 * b + 128 * (tb + 1), 512 * n : 512 * (n + 1)],
                    ys[:],
                )

            def yproj(b):
                yrecv(b)
                for tb in range(2):
                    for n in range(D // 512):
                        ypiece(b, tb, n)

            # ---- schedule -------------------------------------------
            for tc8 in range(NTC):
                qkv_chunk(tc8)

            # wo arrives during attention on the scalar queue
            wo_t = cpool.tile([128, 8 * D], BF16, name="wo_t", tag="wo_t")
            for k in range(8):
                nc.scalar.dma_start(wo_t[:, D * k : D * (k + 1)], wo_d[128 * k : 128 * (k + 1), :])

            for qt in range(NQT):
                attn_group(0, qt)
            nc.gpsimd.collective_compute(
                "AllToAll",
                mybir.AluOpType.bypass,
                replica_groups=[list(range(NC))],
                ins=[ccin[0][:]],
                outs=[ccout[0][:]],
            )
            # b0 output projection spliced into b1 attention: a2a#0 and the
            # Y(b0) matmuls overlap b1 attention in the static PE stream.
            yrecv(0)
            for qt in range(NQT):
                attn_group(1, qt)
                ypiece(0, qt // 2, qt % 2)
            nc.gpsimd.collective_compute(
                "AllToAll",
                mybir.AluOpType.bypass,
                replica_groups=[list(range(NC))],
                ins=[ccin[1][:]],
                outs=[ccout[1][:]],
            )
            yproj(1)

    nc.compile()
    return nc


def _host_inputs(x, Wq, Wk, Wv, Wo):
    x = np.asarray(x, dtype=np.float32)
    Wq, Wk, Wv, Wo = (np.asarray(w, dtype=np.float32) for w in (Wq, Wk, Wv, Wo))
    toks = x.reshape(TOK, D)
    xT = np.ascontiguousarray(toks.T).astype(NPBF16)
    wo_t = np.ascontiguousarray(Wo.T).astype(NPBF16)
    base = 2.0 ** (-8.0 / H)

    cap = np.where(
        np.arange(128)[:, None] <= np.arange(128)[None, :], 3.0e38, -1.0e9
    ).astype(np.float32)
    ind = np.zeros((1, 256), dtype=NPBF16)
    ind[0, 0:64] = 1      # head-A indicator: bc rows 0:64 get denA
    ind[0, 192:256] = 1   # head-B indicator: bc rows 64:128 get denB
    pos_bf = np.arange(T, dtype=np.float32).astype(NPBF16).astype(np.float32)

    in_maps = []
    for c in range(NC):
        hs = slice(128 * c, 128 * (c + 1))
        aug = np.zeros((HPC + 1, T), dtype=NPBF16)
        aug[HPC] = 1
        kbias = np.zeros((128, HPC * NKB), dtype=np.float32)
        for hl in range(HPC):
            h = HPC * c + hl
            slope = base ** (h + 1)
            aug[hl] = (-8.0 * slope * pos_bf).astype(NPBF16)
            for kb in range(NKB):
                kbias[:, NKB * hl + kb] = slope * (128 * kb + np.arange(128))
        in_maps.append(
            {
                "xT": xT,
                "wq": np.ascontiguousarray(Wq[hs, :].T).astype(NPBF16),
                "wk": np.ascontiguousarray(Wk[hs, :].T).astype(NPBF16),
                "wv": np.ascontiguousarray(Wv[hs, :].T).astype(NPBF16),
                "wo": wo_t,
                "aug": aug,
                "kbias": kbias,
                "cap": cap,
                "ind": ind,
            }
        )
    return in_maps


def get_compiled():
    global _COMPILED
    if _COMPILED is None:
        _COMPILED = _build()
    return _COMPILED


def run(x, Wq, Wk, Wv, Wo, trace=False, **trace_kwargs):
    nc = get_compiled()
    in_maps = _host_inputs(x, Wq, Wk, Wv, Wo)
    res = bass_utils.run_bass_kernel_spmd(
        nc, in_maps, core_ids=list(range(NC)), trace=trace, **trace_kwargs
    )
    full = np.empty((TOK, D), dtype=np.float32)
    half = TPC // B  # 256
    for c in range(NC):
        o = res.results[c]["out"]
        full[half * c : half * (c + 1), :] = o[0:half]
        full[T + half * c : T + half * (c + 1), :] = o[half : 2 * half]
    return full.reshape(B, T, D), res


def kernel(x, Wq, Wk, Wv, Wo):
    out, _ = run(x, Wq, Wk, Wv, Wo)
    return out


# revision 17
# speedup vs baseline: 1.2192x; 1.2192x over previous
"""ALiBi causal attention on 8 Trainium2 NeuronCores.

Sharding: tensor-parallel over heads (2 heads/core) for QKV projection and
attention; two batch-split AllToAlls redistribute the (normalized,
transposed) attention outputs so each core owns 256 tokens of each batch
for the output projection. The b0 AllToAll and b0 output projection
overlap with b1's attention compute.

Layout choices (all chosen to avoid on-chip transposes):
  - x is passed host-transposed as xT [D=1024, B*T=4096] in bf16.
  - Q/K are produced in "head-transposed" layout [head_dim, tokens] and
    augmented with one extra contraction row so that the per-query ALiBi
    term -slope*i rides the score matmul (exactly cancelled by softmax,
    so bf16 rounding of it is harmless).
  - Scores are computed transposed: ST[k, q] = K'.T-block @ Q', so the
    softmax reduction (over k) aligns with the AV matmul contraction and
    the denominator falls out of a ones-column appended to V.
  - exp via ScalarE with per-partition bias slope*j in exact f32.
  - Causal masking: only the diagonal-intersecting k-block per q-tile
    needs a 128x128 triangular min-clamp; fully-masked columns are never
    computed or streamed.

Tiles are deliberately small/chunked (xT per [k,512-token] block, Q/K per
[head, 512-token] chunk, V per [token-block]) because Tile's dependency
tracking is per-tile: chunking lets attention start while later
projections still run, and projections start after the first DMA chunk.
DMA queues: xT streams on the sync queue; weights/constants go on the
scalar queue so they don't delay the first projection matmuls.
"""

import sys

if "/opt/trn_rl_repo" not in sys.path:
    sys.path.insert(0, "/opt/trn_rl_repo")

import numpy as np
import ml_dtypes

import concourse.bass as bass
import concourse.bacc as bacc
import concourse.tile as tile
import concourse.mybir as mybir
from concourse import bass_utils

BF16 = mybir.dt.bfloat16
F32 = mybir.dt.float32
NPBF16 = ml_dtypes.bfloat16

B, T, D = 2, 2048, 1024
H, HD = 16, 64
NC = 8
HPC = H // NC          # heads per core = 2
TOK = B * T            # 4096
TPC = TOK // NC        # tokens per core after a2a = 512 (256 per batch)
NKB = T // 128         # 16 k-blocks per sequence
NQT = T // 512         # 4 q-tiles per sequence
NTC = TOK // 512       # 8 token-chunks of 512
KAUG = HD + 1          # 65: head_dim + 1 aug row

_COMPILED = None


def _build():
    nc = bacc.Bacc("TRN2", target_bir_lowering=False, debug=False, num_devices=NC)

    xT_d = nc.dram_tensor("xT", [D, TOK], BF16, kind="ExternalInput")
    wq_d = nc.dram_tensor("wq", [D, 128], BF16, kind="ExternalInput")
    wk_d = nc.dram_tensor("wk", [D, 128], BF16, kind="ExternalInput")
    wv_d = nc.dram_tensor("wv", [D, 128], BF16, kind="ExternalInput")
    wo_d = nc.dram_tensor("wo", [D, D], BF16, kind="ExternalInput")
    aug_d = nc.dram_tensor("aug", [HPC + 1, T], BF16, kind="ExternalInput")
    kbias_d = nc.dram_tensor("kbias", [128, HPC * NKB], F32, kind="ExternalInput")
    cap_d = nc.dram_tensor("cap", [128, 128], F32, kind="ExternalInput")
    ind_d = nc.dram_tensor("ind", [1, 256], BF16, kind="ExternalInput")
    out_d = nc.dram_tensor("out", [TPC, D], F32, kind="ExternalOutput")
    ccin = [
        nc.dram_tensor(f"ccin{b}", [NC * 128, TPC // B], BF16, kind="Internal")
        for b in range(B)
    ]
    ccout = [
        nc.dram_tensor(f"ccout{b}", [NC * 128, TPC // B], BF16, kind="Internal")
        for b in range(B)
    ]

    with tile.TileContext(nc) as tc:
        with (
            tc.tile_pool(name="const", bufs=1) as cpool,
            tc.tile_pool(name="work", bufs=1) as wpool,
            tc.tile_pool(name="ps", bufs=4, space="PSUM") as ps,
            tc.tile_pool(name="psot", bufs=4, space="PSUM") as psot,
        ):
            # ---- xT: one tile per (k-chunk, token-chunk), sync queue --
            xt = [[None] * NTC for _ in range(8)]
            for tc8 in range(NTC):
                for k in range(8):
                    t_ = cpool.tile([128, 512], BF16, name=f"xt{k}_{tc8}", tag=f"xt{k}_{tc8}")
                    nc.sync.dma_start(t_[:], xT_d[128 * k : 128 * (k + 1), 512 * tc8 : 512 * (tc8 + 1)])
                    xt[k][tc8] = t_

            # ---- weights + constants on the scalar DMA queue ----------
            wq_t = cpool.tile([128, D], BF16, name="wq_t", tag="wq_t")
            wk_t = cpool.tile([128, D], BF16, name="wk_t", tag="wk_t")
            wv_t = cpool.tile([128, D], BF16, name="wv_t", tag="wv_t")
            for k in range(8):
                nc.scalar.dma_start(wq_t[:, 128 * k : 128 * (k + 1)], wq_d[128 * k : 128 * (k + 1), :])
                nc.scalar.dma_start(wk_t[:, 128 * k : 128 * (k + 1)], wk_d[128 * k : 128 * (k + 1), :])
                nc.scalar.dma_start(wv_t[:, 128 * k : 128 * (k + 1)], wv_d[128 * k : 128 * (k + 1), :])
            kbias_t = cpool.tile([128, HPC * NKB], F32, name="kbias_t", tag="kbias_t")
            nc.scalar.dma_start(kbias_t[:], kbias_d[:])
            cap_t = cpool.tile([128, 128], F32, name="cap_t", tag="cap_t")
            nc.scalar.dma_start(cap_t[:], cap_d[:])
            ind_t = cpool.tile([1, 256], BF16, name="ind_t", tag="ind_t")
            nc.scalar.dma_start(ind_t[:], ind_d[:])

            # QTa/KTa: per (b, hl, chunk-of-512): [65, 512]; row 64 = aug.
            qta = [[[None] * NQT for _ in range(HPC)] for _ in range(B)]
            kta = [[[None] * NQT for _ in range(HPC)] for _ in range(B)]
            for b in range(B):
                for hl in range(HPC):
                    for c in range(NQT):
                        q_ = cpool.tile([KAUG, 512], BF16, name=f"qta{b}{hl}{c}", tag=f"qta{b}{hl}{c}")
                        k_ = cpool.tile([KAUG, 512], BF16, name=f"kta{b}{hl}{c}", tag=f"kta{b}{hl}{c}")
                        nc.scalar.dma_start(q_[64:65, :], aug_d[hl : hl + 1, 512 * c : 512 * (c + 1)])
                        nc.scalar.dma_start(k_[64:65, :], aug_d[HPC : HPC + 1, 512 * c : 512 * (c + 1)])
                        qta[b][hl][c] = q_
                        kta[b][hl][c] = k_
            # V: per (b, k-block): [128, 130]: 64 cols head A, ones col,
            # 64 cols head B, ones col.
            vt = [[None] * NKB for _ in range(B)]
            for b in range(B):
                for kb in range(NKB):
                    v_ = cpool.tile([128, 130], BF16, name=f"v{b}_{kb}", tag=f"v{b}_{kb}")
                    nc.vector.memset(v_.rearrange("p (a c) -> p a c", c=65)[:, :, 64], 1.0)
                    vt[b][kb] = v_

            # ---- phase 1: QKV projections (chunk-interleaved) ---------
            def qkv_chunk(tc8):
                b, cq = tc8 // NQT, tc8 % NQT
                for w_t, dsts in ((wq_t, qta), (wk_t, kta)):
                    pp = ps.tile([128, 512], F32, name="pp", tag="mm512")
                    for k in range(8):
                        nc.tensor.matmul(
                            pp[:],
                            w_t[:, 128 * k : 128 * (k + 1)],
                            xt[k][tc8][:],
                            start=(k == 0),
                            stop=(k == 7),
                        )
                    nc.vector.tensor_copy(dsts[b][0][cq][0:64, :], pp[0:64, :])
                    nc.vector.tensor_copy(dsts[b][1][cq][0:64, :], pp[64:128, :])
                for j in range(4):
                    kb = 4 * cq + j
                    pv = psot.tile([128, 128], F32, name="pv", tag="otv")
                    for k in range(8):
                        nc.tensor.matmul(
                            pv[:],
                            xt[k][tc8][:, 128 * j : 128 * (j + 1)],
                            wv_t[:, 128 * k : 128 * (k + 1)],
                            start=(k == 0),
                            stop=(k == 7),
                        )
                    nc.vector.tensor_copy(vt[b][kb][:, 0:64], pv[:, 0:64])
                    nc.vector.tensor_copy(vt[b][kb][:, 65:129], pv[:, 64:128])

            # ---- phase 2: attention for one (b, q-tile) ---------------
            def attn_group(b, qt):
                nkb = 4 * qt + 4
                ots = [
                    psot.tile([KAUG, 512], F32, name="ot", tag="otv")
                    for _ in range(HPC)
                ]
                for kb in range(nkb):
                    off = max(0, 128 * (kb - 4 * qt))
                    for hl in range(HPC):
                        sc = ps.tile([128, 512], F32, name="sc", tag="mm512")
                        nc.tensor.matmul(
                            sc[:, off:512],
                            kta[b][hl][kb // 4][:, 128 * (kb % 4) : 128 * (kb % 4 + 1)],
                            qta[b][hl][qt][:, off:512],
                            start=True,
                            stop=True,
                        )
                        if kb >= 4 * qt:
                            nc.vector.tensor_tensor(
                                sc[:, off : off + 128],
                                sc[:, off : off + 128],
                                cap_t[:],
                                mybir.AluOpType.min,
                            )
                        ex = wpool.tile([128, 512], BF16, name="ex", tag="ex", bufs=6)
                        nc.scalar.activation(
                            ex[:, off:512],
                            sc[:, off:512],
                            mybir.ActivationFunctionType.Exp,
                            bias=kbias_t[:, NKB * hl + kb : NKB * hl + kb + 1],
                            scale=0.125,
                        )
                        nc.tensor.matmul(
                            ots[hl][:, off:512],
                            vt[b][kb][:, 65 * hl : 65 * hl + 65],
                            ex[:, off:512],
                            start=(kb == 0),
                            stop=(kb == nkb - 1),
                        )
                dena = wpool.tile([1, 512], BF16, name="dena", tag="dena", bufs=2)
                denb = wpool.tile([1, 512], BF16, name="denb", tag="denb", bufs=2)
                nc.vector.tensor_copy(dena[:], ots[0][64:65, :])
                nc.vector.tensor_copy(denb[:], ots[1][64:65, :])
                bc = ps.tile([128, 512], F32, name="bc", tag="mm512")
                nc.tensor.matmul(bc[:], ind_t[:, 0:128], dena[:], start=True, stop=False)
                nc.tensor.matmul(bc[:], ind_t[:, 128:256], denb[:], start=False, stop=True)
                bcs = wpool.tile([128, 512], F32, name="bcs", tag="bcs", bufs=2)
                nc.vector.tensor_copy(bcs[:], bc[:])
                bci = wpool.tile([128, 512], F32, name="bci", tag="bci", bufs=2)
                nc.vector.reciprocal_approx_fast(bci[:], bcs[:])
                otn = wpool.tile([128, 512], BF16, name="otn", tag="otn", bufs=3)
                nc.vector.tensor_tensor(
                    otn[0:64, :], ots[0][0:64, :], bci[0:64, :], mybir.AluOpType.mult
                )
                nc.vector.tensor_tensor(
                    otn[64:128, :], ots[1][0:64, :], bci[64:128, :], mybir.AluOpType.mult
                )
                # two destination blocks of 256 tokens each
                for half in range(2):
                    blk = 2 * qt + half
                    nc.sync.dma_start(
                        ccin[b][128 * blk : 128 * (blk + 1), :],
                        otn[:, 256 * half : 256 * (half + 1)],
                    )

            # ---- phase 4: output projection for one batch -------------
            at = [[None] * 8 for _ in range(B)]

            def yrecv(b):
                for k in range(8):
                    a_ = cpool.tile([128, TPC // B], BF16, name=f"at{b}_{k}", tag=f"at{b}_{k}")
                    nc.sync.dma_start(a_[:], ccout[b][128 * k : 128 * (k + 1), :])
                    at[b][k] = a_

            def ypiece(b, tb, n):
                yp = ps.tile([128, 512], F32, name="yp", tag="mm512")
                for k in range(8):
                    nc.tensor.matmul(
                        yp[:],
                        at[b][k][:, 128 * tb : 128 * (tb + 1)],
                        wo_t[:, D * k + 512 * n : D * k + 512 * (n + 1)],
                        start=(k == 0),
                        stop=(k == 7),
                    )
                ys = wpool.tile([128, 512], F32, name="ys", tag="ys", bufs=2)
                nc.vector.tensor_copy(ys[:], yp[:])
                nc.sync.dma_start(
                    out_d[256 * b + 128 * tb : 256 * b + 128 * (tb + 1), 512 * n : 512 * (n + 1)],
                    ys[:],
                )

            def yproj(b):
                yrecv(b)
                for tb in range(2):
                    for n in range(D // 512):
                        ypiece(b, tb, n)

            # ---- schedule -------------------------------------------
            for tc8 in range(NTC):
                qkv_chunk(tc8)

            # wo arrives during attention on the scalar queue
            wo_t = cpool.tile([128, 8 * D], BF16, name="wo_t", tag="wo_t")
            for k in range(8):
                nc.scalar.dma_start(wo_t[:, D * k : D * (k + 1)], wo_d[128 * k : 128 * (k + 1), :])

            for qt in range(NQT):
                attn_group(0, qt)
            nc.gpsimd.collective_compute(
                "AllToAll",
                mybir.AluOpType.bypass,
                replica_groups=[list(range(NC))],
                ins=[ccin[0][:]],
                outs=[ccout[0][:]],
            )
            # a2a#0 runs on the collective engine while the PE continues
            # with b1 attention; Y(b0) goes after b1 attention so the static
            # PE stream never stalls on the collective mid-attention, and
            # a2a#1 overlaps the Y(b0) matmuls.
            yrecv(0)
            for qt in range(NQT):
                attn_group(1, qt)
            nc.gpsimd.collective_compute(
                "AllToAll",
                mybir.AluOpType.bypass,
                replica_groups=[list(range(NC))],
                ins=[ccin[1][:]],
                outs=[ccout[1][:]],
            )
            for tb in range(2):
                for n in range(D // 512):
                    ypiece(0, tb, n)
            yproj(1)

    nc.compile()
    return nc


def _host_inputs(x, Wq, Wk, Wv, Wo):
    x = np.asarray(x, dtype=np.float32)
    Wq, Wk, Wv, Wo = (np.asarray(w, dtype=np.float32) for w in (Wq, Wk, Wv, Wo))
    toks = x.reshape(TOK, D)
    xT = np.ascontiguousarray(toks.T).astype(NPBF16)
    wo_t = np.ascontiguousarray(Wo.T).astype(NPBF16)
    base = 2.0 ** (-8.0 / H)

    cap = np.where(
        np.arange(128)[:, None] <= np.arange(128)[None, :], 3.0e38, -1.0e9
    ).astype(np.float32)
    ind = np.zeros((1, 256), dtype=NPBF16)
    ind[0, 0:64] = 1      # head-A indicator: bc rows 0:64 get denA
    ind[0, 192:256] = 1   # head-B indicator: bc rows 64:128 get denB
    pos_bf = np.arange(T, dtype=np.float32).astype(NPBF16).astype(np.float32)

    in_maps = []
    for c in range(NC):
        hs = slice(128 * c, 128 * (c + 1))
        aug = np.zeros((HPC + 1, T), dtype=NPBF16)
        aug[HPC] = 1
        kbias = np.zeros((128, HPC * NKB), dtype=np.float32)
        for hl in range(HPC):
            h = HPC * c + hl
            slope = base ** (h + 1)
            aug[hl] = (-8.0 * slope * pos_bf).astype(NPBF16)
            for kb in range(NKB):
                kbias[:, NKB * hl + kb] = slope * (128 * kb + np.arange(128))
        in_maps.append(
            {
                "xT": xT,
                "wq": np.ascontiguousarray(Wq[hs, :].T).astype(NPBF16),
                "wk": np.ascontiguousarray(Wk[hs, :].T).astype(NPBF16),
                "wv": np.ascontiguousarray(Wv[hs, :].T).astype(NPBF16),
                "wo": wo_t,
                "aug": aug,
                "kbias": kbias,
                "cap": cap,
                "ind": ind,
            }
        )
    return in_maps


def get_compiled():
    global _COMPILED
    if _COMPILED is None:
        _COMPILED = _build()
    return _COMPILED


def run(x, Wq, Wk, Wv, Wo, trace=False, **trace_kwargs):
    nc = get_compiled()
    in_maps = _host_inputs(x, Wq, Wk, Wv, Wo)
    res = bass_utils.run_bass_kernel_spmd(
        nc, in_maps, core_ids=list(range(NC)), trace=trace, **trace_kwargs
    )
    full = np.empty((TOK, D), dtype=np.float32)
    half = TPC // B  # 256
    for c in range(NC):
        o = res.results[c]["out"]
        full[half * c : half * (c + 1), :] = o[0:half]
        full[T + half * c : T + half * (c + 1), :] = o[half : 2 * half]
    return full.reshape(B, T, D), res


def kernel(x, Wq, Wk, Wv, Wo):
    out, _ = run(x, Wq, Wk, Wv, Wo)
    return out
